# revision 31
# baseline (speedup 1.0000x reference)
"""Trainium2 Bass kernel for nn_Conv3DNorm (modulated conv3d + demod + lrelu + clamp).

Reference math (styles == ones):
    dcoef[cout] = rsqrt(sum_{cin,kd,kh,kw} weight^2 + 1e-8)
    y = conv3d(x, weight * dcoef, pad=1)            # per-sample, stride 1
    y = leaky_relu(y + bias, 0.2) * sqrt(2)
    y = clip(y, -256, 256)

Sharding: data-parallel over batch. Core i processes sample i (B=8 == n_cores).
Weight/bias replicated.

v2 layout/engine plan (vs the f32r v1):
  - all matmuls in bf16: walrus emits LDWEIGHTS+MATMUL pairs with FWL and the
    PE hides the weight load behind the previous matmul's streaming -> per-MM
    spacing ~216ns (vs ~242ns for f32r whose 4-byte self-load adds a bubble).
  - x is zero-padded on the HOST to [cin, d, h+2, w+2] so every input DMA is
    contiguous per partition (128 big descriptors instead of 4096x64B) and no
    on-device halo memsets exist at all.
  - demodulation sum-of-squares runs on DVE only (one tensor_tensor_reduce
    over a host-side [cout, cin*27] copy of the weights); v1 used an fp32
    matmul between bf16 FWL matmuls, which corrupted nearby chunks on HW.
  - epilogue: a2 = 0.2*sqrt2*(psum*dcoef+bias); out = clamp(relu(4*a2)+a2)
    == clamp(sqrt2*leaky_relu(psum*dcoef+bias, 0.2)).
"""

import os
import sys

for _p in (
    "/root/.axon_site",
    "/root/.axon_site/_ro/trn_rl_repo",
    "/root/.axon_site/_ro/pypackages",
):
    if os.path.isdir(_p) and _p not in sys.path:
        sys.path.insert(0, _p)

import numpy as np

import concourse.bass as bass  # noqa: F401
import concourse.mybir as mybir
import concourse.tile as tile
from concourse import bacc
from concourse.bass_utils import run_bass_kernel_spmd

# Problem constants (hardcoded per contract).
B = 8
CIN = 128
COUT = 128
D = H = W = 32
K = 3
NTAPS = K * K * K  # 27
HP = H + 2  # 34
WP = W + 2  # 34
NCHUNK = 64  # output chunks of 512 spatial positions: (d, half-of-H)
EPS = 1e-8
S1 = float(np.sqrt(2.0))  # ACT_GAIN * GAIN
CLAMP = 256.0
ALPHA = 0.2

# bisection knobs (defaults = HW-proven config; ttr/8-bufs crashed TRN2)
EPI = os.environ.get("CONV_EPI", "v1")  # "v1" (4-op proven) | "lrelu" (2-op)
PSUM_BUFS = int(os.environ.get("CONV_PSUM_BUFS", "7"))
DEMOD = os.environ.get("CONV_DEMOD", "chain")  # "chain" (mul+reduce) | "ttr"

LAST_RESULTS = None  # BassKernelResults of the most recent run (for test.py)

_CACHED = {}


def _build_nc():
    dt = mybir.dt
    bf16 = dt.bfloat16

    nc = bacc.Bacc("TRN2")
    x_d = nc.dram_tensor("x", [CIN, D, HP, WP], bf16, kind="ExternalInput")
    # x2 = x shifted left one element (host-prepped) so odd-kw taps read
    # 4-byte-aligned rows; misaligned bf16 rows cost ~+6% per matmul.
    x2_d = nc.dram_tensor("x2", [CIN, D, HP, WP], bf16, kind="ExternalInput")
    w_d = nc.dram_tensor("w", [CIN, NTAPS, COUT], bf16, kind="ExternalInput")
    w2_d = nc.dram_tensor("w2", [COUT, NTAPS * CIN], bf16, kind="ExternalInput")
    b2_d = nc.dram_tensor("b2", [COUT, 1], dt.float32, kind="ExternalInput")
    y_d = nc.dram_tensor(
        "y", [COUT, NCHUNK // 2, 1024], dt.float32, kind="ExternalOutput"
    )

    with tile.TileContext(nc) as tc:
        with (
            tc.tile_pool(name="big", bufs=1) as big,
            tc.tile_pool(name="small", bufs=1) as small,
            tc.tile_pool(name="epiv", bufs=3) as vp,
            tc.tile_pool(name="epio", bufs=3) as op,
            tc.tile_pool(name="warmps", bufs=1, space="PSUM") as wps,
        ):
            # ---- PE warmup: zero matmuls bridge the input-DMA window (first
            # real matmul ~15us) so the HAM clock gate reaches 2.4 GHz and,
            # critically, never sees an idle window that re-throttles it.
            wz_l = small.tile([CIN, COUT], bf16)
            nc.vector.memset(wz_l[:], 0.0)
            wz_r = small.tile([CIN, 512], bf16)
            nc.vector.memset(wz_r[:], 0.0)
            ps_warm = wps.tile([COUT, 512], dt.float32)
            for _ in range(16):
                nc.tensor.matmul(ps_warm[:], wz_l[:], wz_r[:], start=True, stop=True)

            # ---- chunk 0 (d=0) only reads kd=1,2 weights and x slices 0,1.
            # Issue those first (sync + scalar HWDGE queues in parallel) so the
            # real matmul stream starts as early as possible. ----
            xpad = big.tile([CIN, D, HP, WP], bf16)
            xpad2 = big.tile([CIN, D, HP, WP], bf16, name="xpad2")
            wk = [big.tile([CIN, 9, COUT], bf16, name=f"wk_{k}") for k in range(K)]
            # Spread chunk-0/1's gating transfers across the three DMA queues
            # (~74GB/s each incl. ramp), ordered by when the tap schedule needs
            # them; partitions 0-63 / 64-127 use disjoint engine sets so
            # half-partition DMAs on two queues ramp in true parallel.
            #   sync:   x_s0 half0, wk1, x_s1 half0   (deadlines 13.3/13.3/15.9us)
            #   scalar: x_s0 half1, wk2, x_s1 half1   (deadlines 13.3/14.6/15.9us)
            #   gpsimd: x2_s0, x2_s1, then the slice stream
            nc.sync.dma_start(xpad[0:64, 0, :, :], x_d[0:64, 0, :, :])
            nc.scalar.dma_start(xpad[64:128, 0, :, :], x_d[64:128, 0, :, :])
            nc.gpsimd.dma_start(xpad2[:, 0, :, :], x2_d[:, 0, :, :])
            nc.sync.dma_start(wk[1][:], w_d[:, 9:18, :])
            nc.scalar.dma_start(wk[2][:], w_d[:, 18:27, :])
            nc.gpsimd.dma_start(xpad2[:, 1, :, :], x2_d[:, 1, :, :])
            nc.sync.dma_start(xpad[0:64, 1, :, :], x_d[0:64, 1, :, :])
            nc.scalar.dma_start(xpad[64:128, 1, :, :], x_d[64:128, 1, :, :])
            nc.sync.dma_start(wk[0][:], w_d[:, 0:9, :])
            for d in range(2, D):
                nc.gpsimd.dma_start(xpad[:, d, :, :], x_d[:, d, :, :])
                nc.gpsimd.dma_start(xpad2[:, d, :, :], x2_d[:, d, :, :])

            # ---- demod scale, DVE/ACT only (no PE involvement) ----
            w2_sb = small.tile([COUT, NTAPS * CIN], bf16)
            nc.sync.dma_start(w2_sb[:], w2_d[:])
            b2_sb = small.tile([COUT, 1], dt.float32)
            nc.sync.dma_start(b2_sb[:], b2_d[:])

            # dscale2 = EPI_GAIN * sqrt2 / sqrt(dsum + eps)
            #         = 1/sqrt(s*dsum + s*eps),  s = 0.5/EPI_GAIN^2
            epi_gain = ALPHA
            s = 0.5 / (epi_gain * epi_gain)
            eps_t = small.tile([COUT, 1], dt.float32)
            nc.vector.memset(eps_t[:], s * EPS)
            dsum = small.tile([COUT, 1], dt.float32)
            if DEMOD == "ttr":
                sq = small.tile([COUT, NTAPS * CIN], dt.float32)
                nc.vector.tensor_tensor_reduce(
                    out=sq[:],
                    in0=w2_sb[:],
                    in1=w2_sb[:],
                    scale=1.0,
                    scalar=0.0,
                    op0=mybir.AluOpType.mult,
                    op1=mybir.AluOpType.add,
                    accum_out=dsum[:],
                )
            else:
                # two standard DVE ops: square then free-dim reduce
                sq = small.tile([COUT, NTAPS * CIN], bf16)
                nc.vector.tensor_mul(sq[:], w2_sb[:], w2_sb[:])
                nc.vector.tensor_reduce(
                    out=dsum[:],
                    in_=sq[:],
                    axis=mybir.AxisListType.X,
                    op=mybir.AluOpType.add,
                )
            srt = small.tile([COUT, 1], dt.float32)
            nc.scalar.activation(
                srt[:],
                dsum[:],
                mybir.ActivationFunctionType.Sqrt,
                scale=s,
                bias=eps_t[:],
            )
            dscale2 = small.tile([COUT, 1], dt.float32)
            nc.vector.reciprocal(dscale2[:], srt[:])
            bias2 = small.tile([COUT, 1], dt.float32)
            nc.scalar.mul(bias2[:], b2_sb[:], epi_gain)

            # ---- main conv loop: 27 accumulated bf16 matmuls per chunk ----
            with tc.tile_pool(name="ps", bufs=PSUM_BUFS, space="PSUM") as psp:
                for c in range(NCHUNK):
                    d, h0 = c // 2, (c % 2) * 16
                    valid = [t for t in range(NTAPS) if 0 <= d + t // 9 - 1 < D]
                    if d == 0:
                        # order chunk-0/1 taps by startup data arrival:
                        # kd=1 plain -> kd=1 shifted -> kd=2 shifted -> kd=2
                        # plain (x_s1 halves are the last gating transfers)
                        valid = (
                            [t for t in valid if t < 18 and t % 3 != 1]
                            + [t for t in valid if t < 18 and t % 3 == 1]
                            + [t for t in valid if t >= 18 and t % 3 == 1]
                            + [t for t in valid if t >= 18 and t % 3 != 1]
                        )
                    # last chunk runs as two asymmetric pieces (320+192 cols)
                    # so the final epilogue+store chain on the tail is short
                    subs = [(0, 16)] if c < NCHUNK - 1 else [(0, 10), (10, 6)]
                    pss = []
                    for s, (hh, nr) in enumerate(subs):
                        ps = psp.tile(
                            [COUT, nr * 32], dt.float32, name=f"ps_{c}_{s}", tag="ps"
                        )
                        pss.append(ps)
                        for t in valid:
                            kd, kh, kw = t // 9, (t // 3) % 3, t % 3
                            r0 = h0 + hh + kh
                            if kw % 2 == 0:
                                rhs = xpad[:, d + kd - 1, r0 : r0 + nr, kw : kw + 32]
                            else:
                                rhs = xpad2[
                                    :, d + kd - 1, r0 : r0 + nr, kw - 1 : kw + 31
                                ]
                            nc.tensor.matmul(
                                ps[:],
                                wk[kd][:, t % 9, :],
                                rhs,
                                start=(t == valid[0]),
                                stop=(t == valid[-1]),
                            )
                    # output stores batched in chunk pairs -> 4KB/partition DMAs
                    if c % 2 == 0:
                        oc2 = op.tile(
                            [COUT, 1024], dt.float32, name=f"oc2_{c//2}", tag="oc2"
                        )
                        oc2_cur = oc2
                    for s, (hh, nr) in enumerate(subs):
                        w = nr * 32
                        base = (c % 2) * 512 + hh * 32
                        ps = pss[s]
                        oc_sub = oc2_cur[:, base : base + w]
                        # proven v1 4-op epilogue:
                        # a2 = 0.2*sqrt2*(ps*dcoef + bias)
                        # out = clamp(relu(4*a2) + a2)
                        a2 = vp.tile(
                            [COUT, w], dt.float32, name=f"a2_{c}_{s}", tag="a2"
                        )
                        nc.vector.tensor_scalar(
                            out=a2[:],
                            in0=ps[:],
                            scalar1=dscale2[:],
                            scalar2=bias2[:],
                            op0=mybir.AluOpType.mult,
                            op1=mybir.AluOpType.add,
                        )
                        r1 = vp.tile(
                            [COUT, w], dt.float32, name=f"r1_{c}_{s}", tag="r1"
                        )
                        nc.scalar.activation(
                            r1[:],
                            a2[:],
                            mybir.ActivationFunctionType.Relu,
                            scale=1.0 / ALPHA - 1.0,
                        )
                        o = op.tile([COUT, w], dt.float32, name=f"o_{c}_{s}", tag="o")
                        nc.vector.scalar_tensor_tensor(
                            out=o[:],
                            in0=r1[:],
                            scalar=1.0,
                            in1=a2[:],
                            op0=mybir.AluOpType.mult,
                            op1=mybir.AluOpType.add,
                        )
                        nc.vector.tensor_scalar(
                            out=oc_sub,
                            in0=o[:],
                            scalar1=-CLAMP,
                            scalar2=CLAMP,
                            op0=mybir.AluOpType.max,
                            op1=mybir.AluOpType.min,
                        )
                        if c >= NCHUNK - 2:
                            # tail chunks: store each piece as soon as it's
                            # clamped so no 512KB store sits on the tail
                            nc.sync.dma_start(
                                y_d[:, c // 2, base : base + w], oc_sub
                            )
                    if c % 2 == 1 and c < NCHUNK - 2:
                        nc.sync.dma_start(y_d[:, c // 2, :], oc2_cur[:])
    nc.compile()
    return nc


def _get_nc():
    key = (EPI, PSUM_BUFS, DEMOD)
    if key not in _CACHED:
        _CACHED[key] = _build_nc()
    return _CACHED[key]


def kernel(x: np.ndarray, weight: np.ndarray, bias: np.ndarray) -> np.ndarray:
    global LAST_RESULTS
    import ml_dtypes

    bf16 = ml_dtypes.bfloat16

    x = np.asarray(x, dtype=np.float32)
    weight = np.asarray(weight, dtype=np.float32)
    bias = np.asarray(bias, dtype=np.float32)

    # [cout, cin, kd, kh, kw] -> [cin, (kd kh kw), cout]  (matmul lhsT layout)
    w_prep = np.ascontiguousarray(
        weight.transpose(1, 2, 3, 4, 0).reshape(CIN, NTAPS, COUT).astype(bf16)
    )
    # [cout, cin*taps] copy for the on-device sum-of-squares (demod)
    w2_prep = np.ascontiguousarray(weight.reshape(COUT, NTAPS * CIN).astype(bf16))
    b2_prep = np.ascontiguousarray((S1 * bias).reshape(COUT, 1).astype(np.float32))

    # host-side zero halo pad in (H, W) so device DMAs are contiguous;
    # x2 = x shifted left one element so odd-kw taps read dword-aligned rows
    xp = np.zeros((B, CIN, D, HP, WP), dtype=bf16)
    xp[:, :, :, 1 : HP - 1, 1 : WP - 1] = x.astype(bf16)
    xp2 = np.zeros((B, CIN, D, HP, WP), dtype=bf16)
    xp2[:, :, :, :, 0 : WP - 1] = xp[:, :, :, :, 1:WP]

    in_maps = [
        {
            "x": xp[i],
            "x2": xp2[i],
            "w": w_prep,
            "w2": w2_prep,
            "b2": b2_prep,
        }
        for i in range(B)
    ]

    nc = _get_nc()
    trace = bool(int(os.environ.get("CONV_TRACE", "0")))
    res = run_bass_kernel_spmd(
        nc,
        in_maps,
        core_ids=list(range(B)),
        trace=trace,
    )
    LAST_RESULTS = res
    out = np.stack(
        [r["y"].reshape(COUT, D, H, W) for r in res.results], axis=0
    ).astype(np.float32)
    return out
